# revision 24
# baseline (speedup 1.0000x reference)
"""Trainium2 Bass kernel: MultiHeadDepthwiseSelfAttention.

Full inputs -> data-parallel over batch across 8 NeuronCores -> full output.

Math (reference):
    q = x*wq + bq; k = x*wk + bk; v = x*wv + bv      (per-channel depthwise)
    att = softmax(q @ k^T / sqrt(F))  per head        (H=12, d=64)
    out = (att @ v) * wo + bo

Algebra used here (per head, channels d in the head):
    S[k,q] = sum_d (xq wq + bq)(xk wk + bk)
           = sum_d u_d xq xk   + cq[q] + ck[k] + c0,   u = wq*wk
  cq[q] is constant along the softmax axis (k) -> drops out of softmax.
  ck[k] + c0 survives only as a per-key factor f[k] = exp((ck+c0)/sqrt(F))
  which is folded into the V rows (incl. the Z ones-column).  With the
  harness's zero biases f == 1 and that path is skipped entirely.
  So Q^T is RAW x^T (host-transposed input, DMA'd) and only the K side
  needs a per-channel scale by u (done on the idle GPSIMD engine).

Kernel strategy (per core, 2 batches):
  - Host folds output projection into v: veff = wv*wo.  beff = bv*wo + bo
    is added at the drain: out = (O_unnorm/Z) + beff since rows of att sum
    to 1.
  - On chip, work transposed: S^T[k,q]; softmax's normalizer Z comes from
    the PV matmul itself via a ones-column per head in Vhat (H*(d+1) cols).
  - QK^T pair is row-tiled on the PE (lhsT base partitions 0/64 -> both
    heads' matmuls run concurrently in the 128x128 array).
  - exp() is split between ACT (exact, LUT) and DVE (Schraudolph bit-trick:
    bits = round(s*A + B) interpreted as fp32; ~3% elementwise, ~3e-3 on
    the final output thanks to numerator/denominator cancellation) to
    balance the two engines; no max-subtraction needed (logits are O(1)).
  - Vhat = x*veff (+ ones cols) on GPSIMD.
  - Unnormalized O^T[d+1, q] is PE-transposed back; one fused DVE
    scalar_tensor_tensor does out = O^T.T * (1/Z) + beff.
"""

import math
import os
import sys

for _p in ("/opt/trn_rl_repo", "/root/.axon_site/_ro/trn_rl_repo"):
    if os.path.isdir(_p) and _p not in sys.path:
        sys.path.insert(0, _p)

import numpy as np

import concourse.bacc as bacc
import concourse.mybir as mybir
from concourse.tile import TileContext
from concourse.masks import make_identity
from concourse.bass_utils import run_bass_kernel_spmd

FP32 = mybir.dt.float32
FP32R = mybir.dt.float32r
BF16 = mybir.dt.bfloat16
I32 = mybir.dt.int32
I16 = mybir.dt.int16
AF = mybir.ActivationFunctionType
ALU = mybir.AluOpType

P = 128
N_CORES = 8
B, N, F, H = 16, 1024, 768, 12

# Schraudolph exp2: exp(s/sqrt(F)) ~ bitcast(int16(s*SCH_A + SCH_B)) as bf16
SCH_C = 0.044  # minimizes max rel err (~3.0%) over the logit range
SCH_A = (1.0 / math.sqrt(F)) * math.log2(math.e) * (1 << 7)
SCH_B = (127.0 - SCH_C) * (1 << 7)


def build(BPC=2, N=N, F=F, H=H, reps=1, loop_reps=None, stages=4,
          mm_dt=BF16, pv_dt=BF16, ptb=4, otb=2, budget=1, dkc=2,
          dve_kcs=((2, 5),), kbias=False, noexp=False,
          novhat=False, spl=640, cpeng="dve", s_dt=FP32, psb=3, otdt=FP32):
    d = F // H            # head dim (64)
    dO = d + 1            # V columns per head incl. ones column
    NT = N // P           # n-tiles (= k-chunks)
    CT = F // P           # channel chunks (== head pairs)
    QB = min(512, N)      # q block (moving-dim) size
    QC = N // QB          # q blocks
    TB = QB // P          # natural q-subtiles per q block
    scale = 1.0 / math.sqrt(F)
    assert P % d == 0 and CT == H // 2

    nc = bacc.Bacc("TRN2", target_bir_lowering=False, debug=False,
                   num_devices=N_CORES)
    x = nc.declare_dram_parameter("x", [BPC, N, F], FP32, isOutput=False)
    xT = nc.declare_dram_parameter("xT", [BPC, F, N], BF16, isOutput=False)
    u = nc.declare_dram_parameter("u", [F], FP32, isOutput=False)
    veff = nc.declare_dram_parameter("veff", [F], FP32, isOutput=False)
    beff = nc.declare_dram_parameter("beff", [F], FP32, isOutput=False)
    if kbias:
        fk = nc.declare_dram_parameter("fk", [BPC, N, H], FP32, isOutput=False)
    out = nc.declare_dram_parameter("out", [BPC, N, F], FP32, isOutput=True)

    with TileContext(nc) as tc:
        with (
            tc.tile_pool(name="const", bufs=1) as cpool,
            tc.tile_pool(name="xp", bufs=2) as xpool,
            tc.tile_pool(name="xtp", bufs=1) as xtpool,
            tc.tile_pool(name="ktp", bufs=1) as ktpool,
            tc.tile_pool(name="vp", bufs=2) as vpool,
            tc.tile_pool(name="op", bufs=1) as opool,
            tc.tile_pool(name="ptp", bufs=ptb) as ptpool,
            tc.tile_pool(name="otp", bufs=otb) as otpool,
            tc.tile_pool(name="rzp", bufs=2) as rzpool,
            tc.tile_pool(name="ps_s", bufs=psb, space="PSUM") as ps_s,
            tc.tile_pool(name="ps_o", bufs=1, space="PSUM") as ps_o,
        ):
            ident = cpool.tile([P, P], FP32)
            make_identity(nc, ident[:])
            identb = cpool.tile([P, P], BF16)
            nc.vector.tensor_copy(out=identb[:], in_=ident[:])
            u_c = cpool.tile([P, CT], FP32)
            veff_b = cpool.tile([P, F], FP32)
            beff_b = cpool.tile([P, F], FP32)

            def emit_consts():
                nc.sync.dma_start(out=veff_b[:],
                                  in_=veff[None, :].broadcast_to([P, F]))
                nc.sync.dma_start(out=beff_b[:],
                                  in_=beff[None, :].broadcast_to([P, F]))
                # u row -> per-partition columns via PE transposes
                row = cpool.tile([1, F], FP32, tag="urow", name="urow")
                nc.sync.dma_start(out=row[:], in_=u[None, :])
                pw = ps_s.tile([P, CT], FP32, tag="ps", name="pw")
                for c in range(CT):
                    nc.tensor.transpose(pw[:, c:c + 1],
                                        row[0:1, c * P:(c + 1) * P],
                                        ident[0:1, 0:1])
                nc.vector.tensor_copy(out=u_c[:], in_=pw[:])

            def emit_vhat(xts, vts, i, fkt=None):
                v3 = vts[i].rearrange("p (h e) -> p h e", e=dO)
                x3 = xts[i].rearrange("p (h e) -> p h e", e=d)
                w3 = veff_b.rearrange("p (h e) -> p h e", e=d)
                nc.gpsimd.memset(vts[i][:, d::dO], 1.0)
                nc.gpsimd.tensor_mul(v3[:, :, 0:d], x3[:], w3[:])
                if fkt is not None:
                    v4 = vts[i].rearrange("p (h e) -> p h e", e=dO)
                    for h in range(H):
                        nc.gpsimd.tensor_scalar_mul(
                            v4[:, h, :], v4[:, h, :], fkt[:, h:h + 1])

            def emit_program(batches):
                NB = len(batches)
                phases = [(bi, c, qc) for bi in range(NB)
                          for c in range(CT) for qc in range(QC)]
                NPH = len(phases)
                pidx = {ph: i for i, ph in enumerate(phases)}

                xts_of, xT_of, kt_of, vts_of, outs_of = {}, {}, {}, {}, {}

                def get_outs(bi):
                    if bi not in outs_of:
                        outs_of[bi] = {
                            i: opool.tile([P, F], FP32, tag=f"on{i}",
                                          name=f"on{i}") for i in range(NT)}
                    return outs_of[bi]

                # work items: (earliest, deadline, fn)
                items = []

                def add_batch_items(bi):
                    xts_of[bi] = [xpool.tile([P, F], FP32, tag=f"xt{i}",
                                             name=f"xt{i}") for i in range(NT)]
                    xT_of[bi] = [xtpool.tile([P, N], mm_dt, tag=f"xT{c}",
                                             name=f"xT{c}") for c in range(CT)]
                    kt_of[bi] = [ktpool.tile([P, N], mm_dt, tag=f"kt{c}",
                                             name=f"kt{c}") for c in range(CT)]
                    vts_of[bi] = [vpool.tile([P, H * dO], pv_dt, tag=f"vt{i}",
                                             name=f"vt{i}") for i in range(NT)]
                    xts, vts = xts_of[bi], vts_of[bi]
                    xTs, kts = xT_of[bi], kt_of[bi]
                    first = pidx[(bi, 0, 0)]
                    ear_load = 0 if bi == 0 else pidx[(bi - 1, 0, 0)]

                    fkts = None
                    if kbias:
                        fkts = [xpool.tile([P, H], FP32, tag=f"fk{i}",
                                           name=f"fk{i}") for i in range(NT)]

                    def xload(bi=bi, xts=xts):
                        for i in range(NT):
                            nc.sync.dma_start(
                                out=xts[i][:],
                                in_=x[batches[bi], i * P:(i + 1) * P, :])
                            if kbias:
                                nc.sync.dma_start(
                                    out=fkts[i][:],
                                    in_=fk[batches[bi], i * P:(i + 1) * P, :])

                    def chunk(c, bi=bi, xTs=xTs, kts=kts):
                        nc.sync.dma_start(
                            out=xTs[c][:],
                            in_=xT[batches[bi], c * P:(c + 1) * P, :])
                        nc.gpsimd.tensor_scalar_mul(kts[c][:], xTs[c][:],
                                                    u_c[:, c:c + 1])

                    if bi == 0:
                        def first_loads():
                            emit_consts()
                            xload()
                            chunk(0)
                        items.append((0, 0, first_loads))
                    else:
                        items.append((ear_load, first, xload))
                        items.append((ear_load, first, lambda: chunk(0)))
                    if not novhat:
                        for i in range(NT):
                            items.append((ear_load, first,
                                          lambda i=i, xts=xts, vts=vts:
                                          emit_vhat(xts, vts, i,
                                                    fkts[i] if kbias else None)))
                    for c in range(1, CT):
                        # chunk c of batch bi: after previous batch stops
                        # reading chunk c, before phase (bi, c, 0)
                        if bi == 0:
                            ear = 0
                        elif c + 1 < CT:
                            ear = pidx[(bi - 1, c + 1, 0)]
                        else:
                            ear = pidx[(bi, 0, 0)]
                        items.append((ear, pidx[(bi, c, 0)],
                                      lambda c=c: chunk(c)))

                for bi in range(NB):
                    add_batch_items(bi)
                items.sort(key=lambda it: (it[0], it[1] if it[1] is not None
                                           else NPH))

                def flush(i, forced_deadline, budget=budget):
                    rest = []
                    n = 0
                    for it in items:
                        ear, dl, fn = it
                        if dl is not None and dl <= forced_deadline:
                            fn()
                        elif ear <= i and n < budget:
                            fn()
                            n += 1
                        else:
                            rest.append(it)
                    items[:] = rest

                # pipeline state
                po2_of, pts_of, pending = {}, {}, [None]

                def emit_s_exp(i, kc):
                    bi, c, qc = phases[i]
                    xTs, kts = xT_of[bi], kt_of[bi]
                    ps = ps_s.tile([P, 2 * QB], s_dt, tag="ps", name="ps")
                    for e in range(2):
                        nc.tensor.matmul(
                            ps[:, e * QB:(e + 1) * QB],
                            lhsT=kts[c][e * d:(e + 1) * d, kc * P:(kc + 1) * P],
                            rhs=xTs[c][e * d:(e + 1) * d, qc * QB:(qc + 1) * QB],
                            start=True, stop=True)
                    if stages < 2:
                        return
                    pt = ptpool.tile([P, 2 * QB], pv_dt, tag="pt", name="pt")
                    if noexp:
                        nc.gpsimd.memset(pt[:, 0:1], 1.0)
                    elif kc in dve_kcs[i % len(dve_kcs)]:
                        nc.vector.tensor_scalar(pt[:].bitcast(I16), ps[:],
                                                SCH_A, SCH_B,
                                                op0=ALU.mult, op1=ALU.add)
                    else:
                        nc.scalar.activation(pt[:], ps[:], AF.Exp, scale=scale)
                    pts_of[(i, kc)] = pt

                def emit_pv(i, kc):
                    if stages < 3:
                        return
                    bi, c, qc = phases[i]
                    vts = vts_of[bi]
                    h0 = 2 * c
                    if i not in po2_of:
                        po2_of[i] = [ps_o.tile([dO, QB], FP32, tag=f"po{e}",
                                               name=f"po{e}") for e in range(2)]
                    pt = pts_of.pop((i, kc))
                    for e in range(2):
                        nc.tensor.matmul(
                            po2_of[i][e][:],
                            lhsT=vts[kc][:, (h0 + e) * dO:(h0 + e + 1) * dO],
                            rhs=pt[:, e * QB:(e + 1) * QB],
                            start=(kc == 0), stop=(kc == NT - 1))

                def emit_drain(i):
                    if stages < 4:
                        return
                    bi, c, qc = phases[i]
                    last_pair = (c == CT - 1)
                    h0 = 2 * c
                    outs = get_outs(bi)
                    po2 = po2_of.pop(i)
                    ots = []
                    for e in range(2):
                        ot = otpool.tile([dO, QB], otdt, tag=f"ot{e}",
                                         name=f"ot{e}")
                        if cpeng == "act":
                            nc.scalar.copy(ot[:], po2[e][:])
                        else:
                            nc.vector.tensor_copy(out=ot[:], in_=po2[e][:])
                        ots.append(ot)

                    def finish():
                        for e in range(2):
                            pn = ps_s.tile([P, TB * dO], FP32, tag="ps",
                                           name="pn")
                            for t in range(TB):
                                nc.tensor.transpose(
                                    pn[:, t * dO:(t + 1) * dO],
                                    ots[e][:, t * P:(t + 1) * P],
                                    (identb if otdt == BF16 else
                                     ident)[0:dO, 0:dO])
                            rz = rzpool.tile([P, TB], FP32, tag="rz", name="rz")
                            nc.vector.reciprocal(rz[:], pn[:, d::dO])
                            ch = (h0 + e) * d
                            for t in range(TB):
                                qsub = qc * TB + t
                                nc.vector.scalar_tensor_tensor(
                                    outs[qsub][:, ch:ch + d],
                                    pn[:, t * dO:t * dO + d],
                                    rz[:, t:t + 1],
                                    beff_b[:, ch:ch + d],
                                    op0=ALU.mult, op1=ALU.add)
                        if last_pair:
                            for t in range(TB):
                                qsub = qc * TB + t
                                nc.sync.dma_start(
                                    out=out[batches[bi],
                                            qsub * P:(qsub + 1) * P, :],
                                    in_=outs[qsub][:])
                    pending[0] = finish

                def flush_pending():
                    if pending[0] is not None:
                        pending[0]()
                        pending[0] = None

                # prologue: phase 0 prefetch
                flush(0, 0, budget=0)
                emit_s_exp(0, 0)
                for i in range(NPH):
                    bi, c, qc = phases[i]
                    for kc in range(1, NT):
                        emit_s_exp(i, kc)
                        if kc == dkc:
                            flush_pending()
                        if kc < NT - 1:
                            emit_pv(i, kc - 1)
                    if i + 1 < NPH:
                        flush(i, i + 1)
                        emit_s_exp(i + 1, 0)
                    emit_pv(i, NT - 2)
                    emit_pv(i, NT - 1)
                    emit_drain(i)
                    if stages >= 4 and (i + 1 == NPH or phases[i + 1][0] != bi):
                        flush_pending()
                        outs_of.pop(bi)
                flush(NPH, NPH, budget=99)

            if loop_reps is None:
                emit_program([bb for _ in range(reps) for bb in range(BPC)])
            else:
                with tc.For_i(0, loop_reps, 1):
                    emit_program(list(range(BPC)))
    nc.compile()
    return nc


_built = {}


def _get_nc(BPC, kbias=False):
    key = (BPC, kbias)
    if key not in _built:
        _built[key] = build(BPC=BPC, kbias=kbias)
    return _built[key]


def host_inputs(x, wq, bq, wk, bk, wv, bv, wo, bo):
    """Host-side folds shared by kernel() and test harnesses."""
    x = np.ascontiguousarray(np.asarray(x, dtype=np.float32))
    wq, bq, wk, bk, wv, bv, wo, bo = (
        np.asarray(t, dtype=np.float32) for t in (wq, bq, wk, bk, wv, bv, wo, bo))
    import ml_dtypes
    xT = np.ascontiguousarray(
        x.transpose(0, 2, 1).astype(ml_dtypes.bfloat16))
    u = wq * wk
    veff = wv * wo
    beff = bv * wo + bo
    kbias = bool(np.any(bq) or np.any(bk))
    fk = None
    if kbias:
        # f[k] = exp((ck[k] + c0)/sqrt(F)) per head; ck[k] = sum_d (bq*wk)_d x[k,d]
        Bx, Nx, Fx = x.shape
        Hd = Fx // H
        bqwk = (bq * wk).reshape(H, Hd)
        c0 = (bq.reshape(H, Hd) * bk.reshape(H, Hd)).sum(-1)
        xh = x.reshape(Bx, Nx, H, Hd)
        ck = np.einsum("bnhd,hd->bnh", xh, bqwk)
        fk = np.exp((ck + c0) / np.sqrt(np.float32(x.shape[2]))).astype(np.float32)
    return x, xT, u, veff, beff, kbias, fk


def kernel(x, wq, bq, wk, bk, wv, bv, wo, bo):
    x, xT, u, veff, beff, kbias, fk = host_inputs(
        x, wq, bq, wk, bk, wv, bv, wo, bo)
    Bx = x.shape[0]
    BPC = Bx // N_CORES
    assert BPC * N_CORES == Bx, (Bx, N_CORES)
    nc = _get_nc(BPC, kbias)
    in_maps = []
    for i in range(N_CORES):
        m = {
            "x": x[i * BPC:(i + 1) * BPC],
            "xT": xT[i * BPC:(i + 1) * BPC],
            "u": u, "veff": veff, "beff": beff,
        }
        if kbias:
            m["fk"] = fk[i * BPC:(i + 1) * BPC]
        in_maps.append(m)
    res = run_bass_kernel_spmd(nc, in_maps, list(range(N_CORES)))
    return np.concatenate([r["out"] for r in res.results], axis=0)


if __name__ == "__main__":
    rng = np.random.default_rng(1)
    inputs = {
        "x": rng.standard_normal((B, N, F), dtype=np.float32),
        "wq": rng.standard_normal((F,), dtype=np.float32),
        "bq": np.zeros(F, np.float32),
        "wk": rng.standard_normal((F,), dtype=np.float32),
        "bk": np.zeros(F, np.float32),
        "wv": rng.standard_normal((F,), dtype=np.float32),
        "bv": np.zeros(F, np.float32),
        "wo": rng.standard_normal((F,), dtype=np.float32),
        "bo": np.zeros(F, np.float32),
    }
    o = kernel(**inputs)
    print("out", o.shape, o.dtype)
